# revision 1
# baseline (speedup 1.0000x reference)
"""Fused transformer decoder layer (self-attn + cross-attn + FFN, 3 LayerNorms)
for Trainium2, SPMD across 8 NeuronCores.

Sharding: 2 cores per batch element (B=4). Each core owns 512 query rows of
its batch element, picked as four 128-row blocks interleaved so the causal
self-attention work is balanced across the pair ({0,3,4,7} / {1,2,5,6}).
K/V projections are computed redundantly per core (no collectives needed).

On-device layout: activations are kept feature-major ("transposed", [D, rows])
so every linear layer uses the weight matrices exactly as stored:
    out^T [Dout, r] = matmul(lhsT=W[D, Dout]-tile, rhs=act^T[D, r]-tile).
Attention scores are computed transposed (scores^T[k, q] = K·Q^T per head);
softmax denominators are harvested by augmenting V with 64 all-ones columns,
which lands the per-query sums partition-replicated next to the attnV output.
The scores/probs path (Q^T, K^T, exp, V_aug) runs in bf16; everything else fp32.
"""

import numpy as np

import concourse.bacc as bacc
import concourse.bass as bass
import concourse.mybir as mybir
import concourse.tile as tile
from concourse.bass_utils import run_bass_kernel_spmd

F32 = mybir.dt.float32
F32R = mybir.dt.float32r
BF16 = mybir.dt.bfloat16
AF = mybir.ActivationFunctionType
ALU = mybir.AluOpType

B, S, D, DFF, H = 4, 1024, 1024, 4096, 16
R = 512                   # query rows per core
NK = D // 128             # 8 k-tiles over D
NP = H // 2               # 8 head pairs
NM2 = DFF // 128          # 32 m-tiles over DFF
EPS = 1e-3
NEG = -1e9
BLOCKS = [[0, 3, 4, 7], [1, 2, 5, 6]]   # 128-row q-blocks per half-core

_NC_CACHE = {}


def _bias_col_ap(t):
    # [N] dram vector -> [128, N//128] (partition-major columns)
    return t.rearrange("(k p) -> p k", p=128)


def _ln(nc, pools, y, gcol, bcol, out_tiles, ones128):
    """LayerNorm over the partition (feature) axis of 8 [128, R] tiles."""
    ps, tmp = pools["psum"], pools["lntmp"]
    pssum = ps.tile([128, R], F32, tag="ps_gen", name="ln_ps_sum")
    for m in range(NK):
        nc.tensor.matmul(pssum, ones128, y[m], start=(m == 0), stop=(m == NK - 1))
    pssq = ps.tile([128, R], F32, tag="ps_gen", name="ln_ps_sq")
    for m in range(NK):
        sq = pools["sq"].tile([128, R], F32R, tag="ln_sqt", name="ln_sqt")
        nc.scalar.activation(sq, y[m].bitcast(F32), AF.Square)
        nc.tensor.matmul(pssq, ones128, sq, start=(m == 0), stop=(m == NK - 1))
    mean = tmp.tile([128, R], F32, tag="ln_mean", name="ln_mean")
    nc.vector.tensor_scalar_mul(mean, pssum, 1.0 / D)
    rv = tmp.tile([128, R], F32, tag="ln_sc", name="ln_rv")
    nc.vector.tensor_scalar_mul(rv, pssq, 1.0 / D)      # E[x^2]
    msq = tmp.tile([128, R], F32, tag="ln_t", name="ln_msq")
    nc.vector.tensor_mul(msq, mean, mean)
    nc.vector.tensor_sub(rv, rv, msq)                   # var
    nc.scalar.activation(rv, rv, AF.Sqrt, bias=pools["epsc"][:, 0:1])
    nc.vector.reciprocal(rv, rv)                        # rstd (broadcast)
    nc.vector.tensor_mul(mean, mean, rv)                # mean*rstd (broadcast)
    for m in range(NK):
        t = tmp.tile([128, R], F32, tag="ln_t", name="ln_t")
        nc.vector.tensor_mul(t, y[m].bitcast(F32), rv)
        nc.vector.tensor_sub(t, t, mean)
        nc.vector.tensor_scalar(out_tiles[m], t, gcol[:, m:m + 1], bcol[:, m:m + 1],
                                ALU.mult, ALU.add)


def _attention(nc, pools, q_src, kv_src, resid, wq, wk, wv, wo,
               bqc, bkc, bv_dram, boc, mask_sb, y_out, ones128r):
    """One multi-head attention block + residual; writes pre-LN y_out tiles."""
    wpool, ps, tmp = pools["w"], pools["psum"], pools["atmp"]
    causal = mask_sb is not None

    # weights stream as column blocks [128, NK, cols] with tag rotation
    wq_cb = wq.rearrange("(k p) (m c) -> m p k c", p=128, c=128)
    wk_cb = wk.rearrange("(k p) (m c) -> m p k c", p=128, c=128)
    wv_cb = wv.rearrange("(k p) (j c) -> j p k c", p=128, c=256)
    wo_cb = wo.rearrange("(k p) (m c) -> m p k c", p=128, c=128)

    # persistent V_aug pair; ones columns written once per attention
    vaugs = [pools["vaug"].tile([128, NK, 256], BF16, tag="vaug",
                                name=f"vaug_{i}") for i in range(2)]
    for i in range(2):
        nc.vector.memset(vaugs[i][:, :, 64:192], 1.0)

    nh = []
    for p in range(NP):
        # ---- V for two pairs produced together (moving dim 256 keeps f32r fast)
        # V_aug[:, s, :]: cols [v_h0(64) | ones(128) | v_h1(64)]
        if p % 2 == 0:
            j = p // 2
            wvb = wpool.tile([128, NK, 256], F32R, tag="wvb", name=f"wvb_{j}")
            nc.sync.dma_start(out=wvb, in_=wv_cb[j])
            bv_chunk = tmp.tile([1, 256], F32R, tag="bv_chunk", name=f"bvc_{j}")
            nc.sync.dma_start(out=bv_chunk,
                              in_=bv_dram.rearrange("(one j c) -> one j c", one=1, c=256)[:, j, :])
            for s in range(NK):
                psv = ps.tile([128, 256], F32, tag="ps_gen", name="ps_v")
                for k in range(NK):
                    nc.tensor.matmul(psv, kv_src[k][:, s * 128:(s + 1) * 128],
                                     wvb[:, k, :],
                                     start=(k == 0), stop=False)
                # bias row via K=1 matmul: psv[m, c] += 1 * bv[c]
                nc.tensor.matmul(psv, ones128r[0:1, :], bv_chunk[0:1, :],
                                 start=False, stop=True)
                for i in range(2):
                    dst = vaugs[i][:, s, :].rearrange(
                        "p (a c) -> p a c", c=64)[:, 0:4:3, :]
                    srcp = psv[:, i * 128:(i + 1) * 128].rearrange(
                        "p (a c) -> p a c", c=64)
                    nc.vector.tensor_copy(dst, srcp)
        vaug = vaugs[p % 2]

        # ---- K^T and Q^T for this pair only (pipelines across pairs) ----
        wcb = wpool.tile([128, NK, 128], F32R, tag="wcb", name=f"wkc_{p}")
        nc.sync.dma_start(out=wcb, in_=wk_cb[p])
        kt_p = pools["kt"].tile([128, S], BF16, tag="kt", name=f"kt_{p}")
        for half in range(2):
            psk = ps.tile([128, R], F32, tag="ps_gen", name="ps_k")
            for k in range(NK):
                nc.tensor.matmul(psk, wcb[:, k, :],
                                 kv_src[k][:, half * 512:(half + 1) * 512],
                                 start=(k == 0), stop=(k == NK - 1))
            nc.scalar.activation(kt_p[:, half * 512:(half + 1) * 512], psk,
                                 AF.Identity, bias=bkc[:, p:p + 1])
        wcb = wpool.tile([128, NK, 128], F32R, tag="wcb", name=f"wqc_{p}")
        nc.sync.dma_start(out=wcb, in_=wq_cb[p])
        psq = ps.tile([128, R], F32, tag="ps_gen", name="ps_q")
        for k in range(NK):
            nc.tensor.matmul(psq, wcb[:, k, :], q_src[k],
                             start=(k == 0), stop=(k == NK - 1))
        qt_p = pools["qt"].tile([128, R], BF16, tag="qt", name=f"qt_{p}")
        nc.scalar.activation(qt_p, psq, AF.Identity, bias=bqc[:, p:p + 1])

        # ---- scores^T, exp, attnV (softmax sums ride along in V_aug ones) ----
        psa0 = pools["psatt"].tile([128, R], F32, tag="ps_att0", name="ps_att0")
        psa1 = pools["psatt"].tile([128, R], F32, tag="ps_att1", name="ps_att1")
        for s in range(NK):
            c0 = 128 * (s // 2) if causal else 0
            n = R - c0
            ssc0 = pools["pssc"].tile([128, R], F32, tag="ps_sc0", name="ps_sc0")
            ssc1 = pools["pssc"].tile([128, R], F32, tag="ps_sc1", name="ps_sc1")
            nc.tensor.matmul(ssc0[:, 0:n], kt_p[0:64, s * 128:(s + 1) * 128],
                             qt_p[0:64, c0:R], start=True, stop=True)
            nc.tensor.matmul(ssc1[:, 0:n], kt_p[64:128, s * 128:(s + 1) * 128],
                             qt_p[64:128, c0:R], start=True, stop=True)
            if causal:
                nc.vector.tensor_add(ssc0[:, 0:128], ssc0[:, 0:128], mask_sb[:, s, :])
                nc.vector.tensor_add(ssc1[:, 0:128], ssc1[:, 0:128], mask_sb[:, s, :])
            e0 = pools["exp"].tile([128, R], BF16, tag="e0", name="e0")
            e1 = pools["exp"].tile([128, R], BF16, tag="e1", name="e1")
            nc.scalar.activation(e0[:, 0:n], ssc0[:, 0:n], AF.Exp, scale=0.125)
            nc.scalar.activation(e1[:, 0:n], ssc1[:, 0:n], AF.Exp, scale=0.125)
            nc.tensor.matmul(psa0[:, c0:R], vaug[:, s, 0:128], e0[:, 0:n],
                             start=(s == 0), stop=(s == NK - 1), skip_group_check=True)
            nc.tensor.matmul(psa1[:, c0:R], vaug[:, s, 128:256], e1[:, 0:n],
                             start=(s == 0), stop=(s == NK - 1), skip_group_check=True)

        # normalize: head0 out rows 0:64 / sums 64:128; head1 sums 0:64 / out 64:128
        nh_p = pools["nh"].tile([128, R], F32R, tag=f"nh_{p}", name=f"nh_{p}")
        inv = tmp.tile([128, R], F32, tag="inv", name="inv")
        invs = tmp.tile([128, R], F32, tag="invs", name="invs")
        nc.vector.reciprocal(inv[64:128, :], psa0[64:128, :])
        nc.sync.dma_start(out=invs[0:64, :], in_=inv[64:128, :])
        nc.vector.tensor_mul(nh_p[0:64, :], psa0[0:64, :], invs[0:64, :])
        nc.vector.reciprocal(inv[0:64, :], psa1[0:64, :])
        nc.sync.dma_start(out=invs[64:128, :], in_=inv[0:64, :])
        nc.vector.tensor_mul(nh_p[64:128, :], psa1[64:128, :], invs[64:128, :])
        nh.append(nh_p)

    # ---- output projection + bias + residual ----
    for m in range(NK):
        wcb = wpool.tile([128, NK, 128], F32R, tag="wcb", name=f"woc_{m}")
        nc.sync.dma_start(out=wcb, in_=wo_cb[m])
        pso = ps.tile([128, R], F32, tag="ps_gen", name="ps_o")
        for p in range(NP):
            nc.tensor.matmul(pso, wcb[:, p, :], nh[p],
                             start=(p == 0), stop=(p == NP - 1))
        nc.vector.scalar_tensor_tensor(y_out[m], pso, boc[:, m:m + 1],
                                       resid[m].bitcast(F32), ALU.add, ALU.add)


def build_nc():
    nc = bacc.Bacc("TRN2", target_bir_lowering=False, debug=False)

    xt = nc.dram_tensor("xt", [D, S], F32R, kind="ExternalInput")
    xq = nc.dram_tensor("xq", [D, R], F32R, kind="ExternalInput")
    enc = nc.dram_tensor("enc", [D, S], F32R, kind="ExternalInput")
    maskst = nc.dram_tensor("maskst", [S, R], BF16, kind="ExternalInput")
    w = {}
    for i in (1, 2):
        for nm in ("wq", "wk", "wv", "wo"):
            w[f"{nm}{i}"] = nc.dram_tensor(f"{nm}{i}", [D, D], F32R, kind="ExternalInput")
        for nm in ("bq", "bk", "bo"):
            w[f"{nm}{i}"] = nc.dram_tensor(f"{nm}{i}", [D], F32, kind="ExternalInput")
        w[f"bv{i}"] = nc.dram_tensor(f"bv{i}", [D], F32R, kind="ExternalInput")
    w["w_ff1"] = nc.dram_tensor("w_ff1", [D, DFF], F32R, kind="ExternalInput")
    w["b_ff1"] = nc.dram_tensor("b_ff1", [DFF], F32, kind="ExternalInput")
    w["w_ff2"] = nc.dram_tensor("w_ff2", [DFF, D], F32R, kind="ExternalInput")
    w["ones_in"] = nc.dram_tensor("ones_in", [128, 128], F32R, kind="ExternalInput")
    w["b_ff2"] = nc.dram_tensor("b_ff2", [D], F32, kind="ExternalInput")
    for i in (1, 2, 3):
        w[f"g{i}"] = nc.dram_tensor(f"g{i}", [D], F32, kind="ExternalInput")
        w[f"be{i}"] = nc.dram_tensor(f"be{i}", [D], F32, kind="ExternalInput")
    out_t = nc.dram_tensor("out_t", [D, R], F32, kind="ExternalOutput")

    from contextlib import ExitStack
    with tile.TileContext(nc) as tc, ExitStack() as ctx:
        pools = {
            "const": ctx.enter_context(tc.tile_pool(name="const", bufs=1)),
            "w": ctx.enter_context(tc.tile_pool(name="wpool", bufs=3)),
            "psum": ctx.enter_context(tc.tile_pool(name="pspool", bufs=2, space="PSUM")),
            "lntmp": ctx.enter_context(tc.tile_pool(name="lntmp", bufs=1)),
            "sq": ctx.enter_context(tc.tile_pool(name="sqpool", bufs=2)),
            "o2p": ctx.enter_context(tc.tile_pool(name="o2pool", bufs=1)),
        }
        const = pools["const"]

        ones128 = const.tile([128, 128], F32R, tag="ones128", name="ones128")
        nc.sync.dma_start(out=ones128, in_=w["ones_in"][:, :])
        epsc = const.tile([128, 1], F32, tag="epsc", name="epsc")
        nc.vector.memset(epsc, EPS)
        pools["epsc"] = epsc
        bias_cols = {}
        for nm in ("bq1", "bk1", "bo1", "bq2", "bk2", "bo2",
                   "b_ff2", "g1", "be1", "g2", "be2", "g3", "be3"):
            t = const.tile([128, NK], F32, tag=f"col_{nm}", name=f"col_{nm}")
            nc.sync.dma_start(out=t, in_=_bias_col_ap(w[nm]))
            bias_cols[nm] = t
        bff1c = const.tile([128, NM2], F32, tag="col_bff1", name="col_bff1")
        nc.sync.dma_start(out=bff1c, in_=_bias_col_ap(w["b_ff1"]))

        o2 = [pools["o2p"].tile([128, R], F32R, tag=f"o2_{m}", name=f"o2_{m}")
              for m in range(NK)]

        # ================= attention scope =================
        with ExitStack() as actx:
            for nm, bufs, space in (("acts", 1, "SBUF"), ("qt", 2, "SBUF"),
                                    ("kt", 2, "SBUF"), ("vaug", 2, "SBUF"),
                                    ("nh", 1, "SBUF"), ("exp", 2, "SBUF"),
                                    ("atmp", 1, "SBUF"), ("amask", 1, "SBUF"),
                                    ("pssc", 2, "PSUM"), ("psatt", 1, "PSUM")):
                pools[nm] = actx.enter_context(
                    tc.tile_pool(name=nm, bufs=(2 if nm == "exp" else bufs), space=space))
            acts = pools["acts"]

            xt_sb = [acts.tile([128, S], F32R, tag=f"kv_{k}", name=f"xt_{k}")
                     for k in range(NK)]
            xq_sb = [acts.tile([128, R], F32R, tag=f"xq_{k}", name=f"xq_{k}")
                     for k in range(NK)]
            for k in range(NK):
                nc.sync.dma_start(out=xt_sb[k], in_=xt[k * 128:(k + 1) * 128, :])
            for k in range(NK):
                nc.sync.dma_start(out=xq_sb[k], in_=xq[k * 128:(k + 1) * 128, :])
            mask_sb = pools["amask"].tile([128, NK, 128], BF16, tag="mask", name="mask")
            for s in range(NK):
                c0 = 128 * (s // 2)
                nc.sync.dma_start(out=mask_sb[:, s, :],
                                  in_=maskst[s * 128:(s + 1) * 128, c0:c0 + 128])

            y1 = [acts.tile([128, R], F32R, tag=f"y_{m}", name=f"y1_{m}")
                  for m in range(NK)]
            _attention(nc, pools, xq_sb, xt_sb, xq_sb,
                       w["wq1"], w["wk1"], w["wv1"], w["wo1"],
                       bias_cols["bq1"], bias_cols["bk1"],
                       w["bv1"], bias_cols["bo1"], mask_sb, y1, ones128)
            enc_sb = [acts.tile([128, S], F32R, tag=f"env_{k}", name=f"enc_{k}")
                      for k in range(NK)]
            for k in range(NK):
                nc.sync.dma_start(out=enc_sb[k], in_=enc[k * 128:(k + 1) * 128, :])
            o1 = [acts.tile([128, R], F32R, tag=f"xq_{m}", name=f"o1_{m}")
                  for m in range(NK)]
            _ln(nc, pools, y1, bias_cols["g1"], bias_cols["be1"], o1, ones128)

            y2 = [acts.tile([128, R], F32R, tag=f"y_{m}", name=f"y2_{m}")
                  for m in range(NK)]
            _attention(nc, pools, o1, enc_sb, o1,
                       w["wq2"], w["wk2"], w["wv2"], w["wo2"],
                       bias_cols["bq2"], bias_cols["bk2"],
                       w["bv2"], bias_cols["bo2"], None, y2, ones128)
            _ln(nc, pools, y2, bias_cols["g2"], bias_cols["be2"], o2, ones128)

        # ================= FFN scope =================
        with ExitStack() as fctx:
            hpool = fctx.enter_context(tc.tile_pool(name="hpool", bufs=1))
            facts = fctx.enter_context(tc.tile_pool(name="facts", bufs=1))
            o3p = fctx.enter_context(tc.tile_pool(name="o3pool", bufs=2))

            wff1_ap = w["w_ff1"].rearrange("(k p) (m c) -> m p k c", p=128, c=128)
            h = []
            for m in range(NM2):
                wt = pools["w"].tile([128, NK, 128], F32R, tag="wcb",
                                     name=f"wff1_{m}")
                nc.sync.dma_start(out=wt, in_=wff1_ap[m])
                psh = pools["psum"].tile([128, R], F32, tag="ps_gen", name="ps_h")
                for k in range(NK):
                    nc.tensor.matmul(psh, wt[:, k, :], o2[k],
                                     start=(k == 0), stop=(k == NK - 1))
                h_m = hpool.tile([128, R], F32R, tag=f"h_{m}", name=f"h_{m}")
                nc.scalar.activation(h_m, psh, AF.Relu, bias=bff1c[:, m:m + 1])
                h.append(h_m)

            wff2_ap = w["w_ff2"].rearrange("(q k p) (m c) -> m q p k c",
                                           p=128, c=128, k=NK)
            y3 = [facts.tile([128, R], F32R, tag=f"y3_{m}", name=f"y3_{m}")
                  for m in range(NK)]
            for m in range(NK):
                psf = pools["psum"].tile([128, R], F32, tag="ps_gen", name="ps_f")
                for q in range(NM2 // NK):
                    wt = pools["w"].tile([128, NK, 128], F32R, tag="wcb",
                                         name=f"wff2_{m}_{q}")
                    nc.sync.dma_start(out=wt, in_=wff2_ap[m, q])
                    for k in range(NK):
                        nc.tensor.matmul(psf, wt[:, k, :], h[q * NK + k],
                                         start=(q == 0 and k == 0),
                                         stop=(q == NM2 // NK - 1 and k == NK - 1))
                nc.vector.scalar_tensor_tensor(y3[m], psf, bias_cols["b_ff2"][:, m:m + 1],
                                               o2[m].bitcast(F32), ALU.add, ALU.add)
            o3 = [o3p.tile([128, R], F32, tag="o3", name=f"o3_{m}")
                  for m in range(NK)]
            _ln(nc, pools, y3, bias_cols["g3"], bias_cols["be3"], o3, ones128)
            for m in range(NK):
                nc.sync.dma_start(out=out_t[m * 128:(m + 1) * 128, :], in_=o3[m])

    nc.compile()
    return nc


def _get_nc():
    if "nc" not in _NC_CACHE:
        _NC_CACHE["nc"] = build_nc()
    return _NC_CACHE["nc"]


def _make_in_maps(inputs):
    full_k = np.arange(S)
    shared = {}
    for nm in ("wq1", "wk1", "wv1", "wo1", "wq2", "wk2", "wv2", "wo2",
               "bq1", "bk1", "bv1", "bo1", "bq2", "bk2", "bv2", "bo2",
               "w_ff1", "b_ff1", "w_ff2", "b_ff2",
               "g1", "be1", "g2", "be2", "g3", "be3"):
        shared[nm] = np.ascontiguousarray(inputs[nm], dtype=np.float32)
    in_maps = []
    metas = []
    for c in range(8):
        b, half = c // 2, c % 2
        qidx = np.concatenate([np.arange(128) + 128 * blk for blk in BLOCKS[half]])
        xt_b = np.ascontiguousarray(np.asarray(inputs["inputs"][b]).T.astype(np.float32))
        enc_b = np.ascontiguousarray(np.asarray(inputs["enc_outputs"][b]).T.astype(np.float32))
        xq_b = np.ascontiguousarray(xt_b[:, qidx])
        import ml_dtypes
        mask = np.where(full_k[:, None] <= qidx[None, :], 0.0, NEG).astype(ml_dtypes.bfloat16)
        m = dict(shared)
        m.update({"xt": xt_b, "xq": xq_b, "enc": enc_b,
                  "maskst": np.ascontiguousarray(mask),
                  "ones_in": np.ones((128, 128), dtype=np.float32)})
        in_maps.append(m)
        metas.append((b, qidx))
    return in_maps, metas


def kernel(**inputs):
    nc = _get_nc()
    in_maps, metas = _make_in_maps(inputs)
    res = run_bass_kernel_spmd(nc, in_maps, core_ids=list(range(8)))
    out = np.zeros((B, S, D), dtype=np.float32)
    for c, (b, qidx) in enumerate(metas):
        out[b, qidx, :] = res.results[c]["out_t"].T
    return out



# revision 20
# speedup vs baseline: 1.8435x; 1.8435x over previous
"""Fused transformer decoder layer (self-attn + cross-attn + FFN, 3 LayerNorms)
for Trainium2, SPMD across 8 NeuronCores.

Sharding: 2 cores per batch element (B=4). Each core owns 512 query rows of
its batch element, picked as four 128-row blocks interleaved so the causal
self-attention work is balanced across the pair ({0,3,4,7} / {1,2,5,6}).
K/V projections are computed redundantly per core (no collectives needed).

On-device layout: activations are kept feature-major ("transposed", [D, rows])
so every linear layer uses the weight matrices exactly as stored:
    out^T [Dout, r] = matmul(lhsT=W[D, Dout]-tile, rhs=act^T[D, r]-tile).
Attention scores are computed transposed (scores^T[k, q] = K.Q^T per head);
softmax denominators are harvested by augmenting V with 64 all-ones columns,
which lands the per-query sums partition-replicated next to the attnV output.

All matmul operands are bf16 (fp32 PSUM accumulation); LayerNorm statistics
run in fp32. Host-side, the inputs are packed into just 3 DRAM tensors to
minimize per-call dispatch overhead:
  apack [D, 3584] bf16 : xt^T | xq^T | enc^T | causal-mask blocks
  wpack [128, 133120] bf16 : all 10 weight matrices pre-tiled so every
                             weight DMA is a dense [128, ncols] column slice
  cpack [128, 136] f32 : bias / gamma / beta columns
"""

import numpy as np

import concourse.bacc as bacc
import concourse.bass as bass
import concourse.mybir as mybir
import concourse.tile as tile
from concourse.bass_utils import run_bass_kernel_spmd

F32 = mybir.dt.float32
F32R = mybir.dt.float32r
BF16 = mybir.dt.bfloat16
AF = mybir.ActivationFunctionType
ALU = mybir.AluOpType

B, S, D, DFF, H = 4, 1024, 1024, 4096, 16
R = 512                   # query rows per core
NK = D // 128             # 8 k-tiles over D
NP = H // 2               # 8 head pairs
NM2 = DFF // 128          # 32 m-tiles over DFF
EPS = 1e-3
NEG = -1e9
BLOCKS = [[0, 3, 4, 7], [1, 2, 5, 6]]   # 128-row q-blocks per half-core

# ---- packed-layout offsets (columns) ----
# apack columns
A_XT, A_XQ, A_ENC, A_MASK = 0, 1024, 1536, 2560
A_COLS = 3584
# wpack columns: each D x D weight is 8 tiles x 1024 cols (wv: 4 x 2048)
WOFF = {"wq1": 0, "wk1": 8192, "wv1": 16384, "wo1": 24576,
        "wq2": 32768, "wk2": 40960, "wv2": 49152, "wo2": 57344,
        "w_ff1": 65536, "w_ff2": 98304, "bv1": 131072, "bv2": 132096}
W_COLS = 133120
# cpack columns
COFF = {"bq1": 0, "bk1": 8, "bo1": 16, "bq2": 24, "bk2": 32, "bo2": 40,
        "b_ff2": 48, "g1": 56, "be1": 64, "g2": 72, "be2": 80,
        "g3": 88, "be3": 96, "b_ff1": 104}
C_COLS = 136

_NC_CACHE = {}


def _ln(nc, pools, y, bc, g_off, be_off, out_tiles, ones128):
    """LayerNorm over the partition (feature) axis of 8 [128, R] f32 tiles."""
    ps, tmp = pools["psum"], pools["lntmp"]
    pssum = ps.tile([128, R], F32, tag="ps_gen", name="ln_ps_sum")
    for m in range(NK):
        nc.tensor.matmul(pssum, ones128, y[m], start=(m == 0), stop=(m == NK - 1))
    pssq = ps.tile([128, R], F32, tag="ps_gen", name="ln_ps_sq")
    for m in range(NK):
        sq = pools["sq"].tile([128, R], F32R, tag="ln_sqt", name="ln_sqt")
        nc.scalar.activation(sq, y[m].bitcast(F32), AF.Square)
        nc.tensor.matmul(pssq, ones128, sq, start=(m == 0), stop=(m == NK - 1))
    mean = tmp.tile([128, R], F32, tag="ln_mean", name="ln_mean")
    nc.vector.tensor_scalar_mul(mean, pssum, 1.0 / D)
    rv = tmp.tile([128, R], F32, tag="ln_sc", name="ln_rv")
    nc.vector.tensor_scalar_mul(rv, pssq, 1.0 / D)      # E[x^2]
    msq = tmp.tile([128, R], F32, tag="ln_t", name="ln_msq")
    nc.vector.tensor_mul(msq, mean, mean)
    nc.vector.tensor_sub(rv, rv, msq)                   # var
    nc.scalar.activation(rv, rv, AF.Sqrt, bias=pools["epsc"][:, 0:1])
    nc.vector.reciprocal(rv, rv)                        # rstd (broadcast)
    nc.vector.tensor_mul(mean, mean, rv)                # mean*rstd (broadcast)
    for m in range(NK):
        t = tmp.tile([128, R], F32, tag="ln_t", name="ln_t")
        nc.vector.tensor_mul(t, y[m].bitcast(F32), rv)
        nc.vector.tensor_sub(t, t, mean)
        nc.vector.tensor_scalar(out_tiles[m], t, bc[:, g_off + m:g_off + m + 1],
                                bc[:, be_off + m:be_off + m + 1],
                                ALU.mult, ALU.add)


def _attention(nc, pools, wpack, bc, q_src, kv_src, resid, wq_off, wk_off,
               wv_off, wo_off, bq_off, bk_off, bv_off, bo_off,
               mask_sb, y_out, ones_bv):
    """One multi-head attention block + residual; writes pre-LN y_out tiles.

    q_src/kv_src/resid: bf16 [128, *] tiles. y_out: f32 [128, R] tiles.
    """
    wpool, ps, tmp = pools["w"], pools["psum"], pools["atmp"]
    causal = mask_sb is not None

    # persistent V_aug tiles (one per pair in the current 4-pair group);
    # ones columns written once per attention
    vaugs = [pools["vaug"].tile([128, NK, 256], BF16, tag=f"vaug_{i}",
                                name=f"vaug_{i}") for i in range(4)]
    for i in range(4):
        nc.vector.memset(vaugs[i][:, :, 64:192], 1.0)

    nh = []
    for p in range(NP):
        # ---- V for four head pairs produced together (moving dim 512) ----
        # V_aug[:, s, :]: cols [v_h0(64) | ones(128) | v_h1(64)]
        if p % 4 == 0:
            g = p // 4
            wvb = wpool.tile([128, 4096], BF16, tag="wvb", name=f"wvb_{g}")
            nc.sync.dma_start(out=wvb,
                              in_=wpack[:, wv_off + g * 4096:wv_off + (g + 1) * 4096])
            bv_chunk = tmp.tile([1, 512], BF16, tag="bv_chunk", name=f"bvc_{g}")
            nc.sync.dma_start(out=bv_chunk,
                              in_=wpack[0:1, bv_off + g * 512:bv_off + (g + 1) * 512])
            # partition-replicated bias tile, one K=1 matmul per group
            psB = ps.tile([128, 512], F32, tag="ps_gen", name="ps_bv")
            nc.tensor.matmul(psB, ones_bv[0:1, :], bv_chunk[0:1, :],
                             start=True, stop=True)
            brep = tmp.tile([128, 512], F32, tag="brep", name=f"brep_{g}")
            nc.vector.tensor_copy(brep, psB)
            for s in range(NK):
                psv = ps.tile([128, 512], F32, tag="ps_gen", name="ps_v")
                for k in range(NK):
                    nc.tensor.matmul(psv, kv_src[k][:, s * 128:(s + 1) * 128],
                                     wvb[:, k * 512:(k + 1) * 512],
                                     start=(k == 0), stop=(k == NK - 1))
                for i in range(4):
                    dst = vaugs[i][:, s, :].rearrange(
                        "p (a c) -> p a c", c=64)[:, 0:4:3, :]
                    srcp = psv[:, i * 128:(i + 1) * 128].rearrange(
                        "p (a c) -> p a c", c=64)
                    brp = brep[:, i * 128:(i + 1) * 128].rearrange(
                        "p (a c) -> p a c", c=64)
                    nc.vector.tensor_tensor(dst, srcp, brp, ALU.add)
        vaug = vaugs[p % 4]

        # ---- Q^T then K^T for this pair (ACT latencies hide under K matmuls) ----
        wcb = wpool.tile([128, 1024], BF16, tag="wcb", name=f"wqc_{p}")
        nc.sync.dma_start(out=wcb,
                          in_=wpack[:, wq_off + p * 1024:wq_off + (p + 1) * 1024])
        psq = ps.tile([128, R], F32, tag="ps_gen", name="ps_q")
        for k in range(NK):
            nc.tensor.matmul(psq, wcb[:, k * 128:(k + 1) * 128], q_src[k],
                             start=(k == 0), stop=(k == NK - 1))
        qt_p = pools["qt"].tile([128, R], BF16, tag="qt", name=f"qt_{p}")
        nc.scalar.activation(qt_p, psq, AF.Identity,
                             bias=bc[:, bq_off + p:bq_off + p + 1])
        wcb = wpool.tile([128, 1024], BF16, tag="wcb", name=f"wkc_{p}")
        nc.sync.dma_start(out=wcb,
                          in_=wpack[:, wk_off + p * 1024:wk_off + (p + 1) * 1024])
        kt_p = pools["kt"].tile([128, S], BF16, tag="kt", name=f"kt_{p}")
        psk0 = ps.tile([128, R], F32, tag="ps_gen", name="ps_k0")
        psk1 = ps.tile([128, R], F32, tag="ps_gen", name="ps_k1")
        for k in range(NK):
            # both halves share the stationary -> one weight load per k
            nc.tensor.matmul(psk0, wcb[:, k * 128:(k + 1) * 128],
                             kv_src[k][:, 0:512],
                             start=(k == 0), stop=(k == NK - 1),
                             skip_group_check=True)
            nc.tensor.matmul(psk1, wcb[:, k * 128:(k + 1) * 128],
                             kv_src[k][:, 512:1024],
                             start=(k == 0), stop=(k == NK - 1),
                             skip_group_check=True)
        nc.scalar.activation(kt_p[:, 0:512], psk0,
                             AF.Identity, bias=bc[:, bk_off + p:bk_off + p + 1])
        nc.scalar.activation(kt_p[:, 512:1024], psk1,
                             AF.Identity, bias=bc[:, bk_off + p:bk_off + p + 1])

        # ---- scores^T, exp, attnV (softmax sums ride along in V_aug ones) ----
        psa0 = pools["psatt"].tile([128, R], F32, tag="ps_att0", name="ps_att0")
        psa1 = pools["psatt"].tile([128, R], F32, tag="ps_att1", name="ps_att1")
        pend = None
        for s in range(NK):
            c0 = 128 * (s // 2) if causal else 0
            n = R - c0
            ssc0 = pools["pssc"].tile([128, R], F32, tag="ps_sc0", name="ps_sc0")
            ssc1 = pools["pssc"].tile([128, R], F32, tag="ps_sc1", name="ps_sc1")
            nc.tensor.matmul(ssc0[:, 0:n], kt_p[0:64, s * 128:(s + 1) * 128],
                             qt_p[0:64, c0:R], start=True, stop=True)
            nc.tensor.matmul(ssc1[:, 0:n], kt_p[64:128, s * 128:(s + 1) * 128],
                             qt_p[64:128, c0:R], start=True, stop=True)
            if causal:
                nc.vector.tensor_add(ssc0[:, 0:128], ssc0[:, 0:128],
                                     mask_sb[:, s * 128:(s + 1) * 128])
                nc.vector.tensor_add(ssc1[:, 0:128], ssc1[:, 0:128],
                                     mask_sb[:, s * 128:(s + 1) * 128])
            e0 = pools["exp"].tile([128, R], BF16, tag="e0", name="e0")
            e1 = pools["exp"].tile([128, R], BF16, tag="e1", name="e1")
            nc.scalar.activation(e0[:, 0:n], ssc0[:, 0:n], AF.Exp, scale=0.125)
            nc.scalar.activation(e1[:, 0:n], ssc1[:, 0:n], AF.Exp, scale=0.125)
            if pend is not None:
                pc0, pn, pe0, pe1, pss = pend
                nc.tensor.matmul(psa0[:, pc0:R], vaug[:, pss, 0:128], pe0[:, 0:pn],
                                 start=(pss == 0), stop=False, skip_group_check=True)
                nc.tensor.matmul(psa1[:, pc0:R], vaug[:, pss, 128:256], pe1[:, 0:pn],
                                 start=(pss == 0), stop=False, skip_group_check=True)
            pend = (c0, n, e0, e1, s)
        pc0, pn, pe0, pe1, pss = pend
        nc.tensor.matmul(psa0[:, pc0:R], vaug[:, pss, 0:128], pe0[:, 0:pn],
                         start=False, stop=True, skip_group_check=True)
        nc.tensor.matmul(psa1[:, pc0:R], vaug[:, pss, 128:256], pe1[:, 0:pn],
                         start=False, stop=True, skip_group_check=True)

        # normalize: head0 out rows 0:64 / sums 64:128; head1 sums 0:64 / out 64:128
        nh_p = pools["nh"].tile([128, R], BF16, tag=f"nh_{p}", name=f"nh_{p}")
        inv = tmp.tile([128, R], F32, tag="inv", name="inv")
        invs = tmp.tile([128, R], F32, tag="invs", name="invs")
        nc.vector.reciprocal(inv[64:128, :], psa0[64:128, :])
        nc.sync.dma_start(out=invs[0:64, :], in_=inv[64:128, :])
        nc.vector.tensor_mul(nh_p[0:64, :], psa0[0:64, :], invs[0:64, :])
        nc.vector.reciprocal(inv[0:64, :], psa1[0:64, :])
        nc.sync.dma_start(out=invs[64:128, :], in_=inv[0:64, :])
        nc.vector.tensor_mul(nh_p[64:128, :], psa1[64:128, :], invs[64:128, :])
        nh.append(nh_p)

    # ---- output projection + bias + residual ----
    for m in range(NK):
        wcb = wpool.tile([128, 1024], BF16, tag="wcb", name=f"woc_{m}")
        nc.sync.dma_start(out=wcb,
                          in_=wpack[:, wo_off + m * 1024:wo_off + (m + 1) * 1024])
        pso = ps.tile([128, R], F32, tag="ps_gen", name="ps_o")
        for p in range(NP):
            nc.tensor.matmul(pso, wcb[:, p * 128:(p + 1) * 128], nh[p],
                             start=(p == 0), stop=(p == NP - 1))
        nc.vector.scalar_tensor_tensor(y_out[m], pso, bc[:, bo_off + m:bo_off + m + 1],
                                       resid[m], ALU.add, ALU.add)


def build_nc():
    nc = bacc.Bacc("TRN2", target_bir_lowering=False, debug=False)

    apack = nc.dram_tensor("apack", [D, A_COLS], BF16, kind="ExternalInput")
    wpack = nc.dram_tensor("wpack", [128, W_COLS], BF16, kind="ExternalInput")
    cpack = nc.dram_tensor("cpack", [128, C_COLS], F32, kind="ExternalInput")
    out_t = nc.dram_tensor("out_t", [D, R], F32, kind="ExternalOutput")

    from contextlib import ExitStack
    with tile.TileContext(nc) as tc, ExitStack() as ctx:
        pools = {
            "const": ctx.enter_context(tc.tile_pool(name="const", bufs=1)),
            "w": ctx.enter_context(tc.tile_pool(name="wpool", bufs=4)),
            "psum": ctx.enter_context(tc.tile_pool(name="pspool", bufs=2, space="PSUM")),
            "lntmp": ctx.enter_context(tc.tile_pool(name="lntmp", bufs=1)),
            "sq": ctx.enter_context(tc.tile_pool(name="sqpool", bufs=2)),
            "o2p": ctx.enter_context(tc.tile_pool(name="o2pool", bufs=1)),
        }
        const = pools["const"]

        ones_f = const.tile([128, 128], F32, tag="ones_f", name="ones_f")
        nc.vector.memset(ones_f, 1.0)
        ones128 = const.tile([128, 128], F32R, tag="ones128", name="ones128")
        nc.vector.tensor_copy(ones128, ones_f)
        ones_bv = const.tile([128, 128], BF16, tag="ones_bv", name="ones_bv")
        nc.vector.memset(ones_bv, 1.0)
        epsc = const.tile([128, 1], F32, tag="epsc", name="epsc")
        nc.vector.memset(epsc, EPS)
        pools["epsc"] = epsc
        bc = const.tile([128, C_COLS], F32, tag="bc", name="bc")
        nc.sync.dma_start(out=bc, in_=cpack[:, :])

        o2 = [pools["o2p"].tile([128, R], BF16, tag=f"o2_{m}", name=f"o2_{m}")
              for m in range(NK)]

        # ================= attention scope =================
        with ExitStack() as actx:
            for nm, bufs, space in (("acts", 1, "SBUF"), ("qt", 2, "SBUF"),
                                    ("kt", 2, "SBUF"), ("vaug", 2, "SBUF"),
                                    ("nh", 1, "SBUF"), ("exp", 2, "SBUF"),
                                    ("atmp", 1, "SBUF"), ("amask", 1, "SBUF"),
                                    ("pssc", 2, "PSUM"), ("psatt", 1, "PSUM")):
                pools[nm] = actx.enter_context(
                    tc.tile_pool(name=nm, bufs=bufs, space=space))
            acts = pools["acts"]

            xt_sb = [acts.tile([128, S], BF16, tag=f"kv_{k}", name=f"xt_{k}")
                     for k in range(NK)]
            xq_sb = [acts.tile([128, R], BF16, tag=f"xq_{k}", name=f"xq_{k}")
                     for k in range(NK)]
            for k in range(NK):
                nc.sync.dma_start(out=xt_sb[k][:, 0:512],
                                  in_=apack[k * 128:(k + 1) * 128, A_XT:A_XT + 512])
            for k in range(NK):
                nc.sync.dma_start(out=xt_sb[k][:, 512:1024],
                                  in_=apack[k * 128:(k + 1) * 128, A_XT + 512:A_XT + S])
            for k in range(NK):
                nc.sync.dma_start(out=xq_sb[k],
                                  in_=apack[k * 128:(k + 1) * 128, A_XQ:A_XQ + R])
            mask_sb = pools["amask"].tile([128, 1024], BF16, tag="mask", name="mask")
            nc.sync.dma_start(out=mask_sb, in_=apack[0:128, A_MASK:A_MASK + 1024])

            y1 = [acts.tile([128, R], F32R, tag=f"y_{m}", name=f"y1_{m}")
                  for m in range(NK)]
            _attention(nc, pools, wpack, bc, xq_sb, xt_sb, xq_sb,
                       WOFF["wq1"], WOFF["wk1"], WOFF["wv1"], WOFF["wo1"],
                       COFF["bq1"], COFF["bk1"], WOFF["bv1"], COFF["bo1"],
                       mask_sb, y1, ones_bv)
            enc_sb = [acts.tile([128, S], BF16, tag=f"env_{k}", name=f"enc_{k}")
                      for k in range(NK)]
            for k in range(NK):
                nc.sync.dma_start(out=enc_sb[k],
                                  in_=apack[k * 128:(k + 1) * 128, A_ENC:A_ENC + S])
            o1 = [acts.tile([128, R], BF16, tag=f"xq_{m}", name=f"o1_{m}")
                  for m in range(NK)]
            _ln(nc, pools, y1, bc, COFF["g1"], COFF["be1"], o1, ones128)

            y2 = [acts.tile([128, R], F32R, tag=f"y_{m}", name=f"y2_{m}")
                  for m in range(NK)]
            _attention(nc, pools, wpack, bc, o1, enc_sb, o1,
                       WOFF["wq2"], WOFF["wk2"], WOFF["wv2"], WOFF["wo2"],
                       COFF["bq2"], COFF["bk2"], WOFF["bv2"], COFF["bo2"],
                       None, y2, ones_bv)
            _ln(nc, pools, y2, bc, COFF["g2"], COFF["be2"], o2, ones128)

        # ================= FFN scope =================
        with ExitStack() as fctx:
            hpool = fctx.enter_context(tc.tile_pool(name="hpool", bufs=1))
            facts = fctx.enter_context(tc.tile_pool(name="facts", bufs=1))
            o3p = fctx.enter_context(tc.tile_pool(name="o3pool", bufs=2))

            ff1 = WOFF["w_ff1"]
            h = []
            for m in range(NM2):
                wt = pools["w"].tile([128, 1024], BF16, tag="wcb",
                                     name=f"wff1_{m}")
                nc.sync.dma_start(out=wt,
                                  in_=wpack[:, ff1 + m * 1024:ff1 + (m + 1) * 1024])
                psh = pools["psum"].tile([128, R], F32, tag="ps_gen", name="ps_h")
                for k in range(NK):
                    nc.tensor.matmul(psh, wt[:, k * 128:(k + 1) * 128], o2[k],
                                     start=(k == 0), stop=(k == NK - 1))
                h_m = hpool.tile([128, R], BF16, tag=f"h_{m}", name=f"h_{m}")
                nc.scalar.activation(h_m, psh, AF.Relu,
                                     bias=bc[:, COFF["b_ff1"] + m:COFF["b_ff1"] + m + 1])
                h.append(h_m)

            ff2 = WOFF["w_ff2"]
            y3 = [facts.tile([128, R], F32R, tag=f"y3_{m}", name=f"y3_{m}")
                  for m in range(NK)]
            for m in range(NK):
                psf = pools["psum"].tile([128, R], F32, tag="ps_gen", name="ps_f")
                for q in range(NM2 // NK):
                    wt = pools["w"].tile([128, 1024], BF16, tag="wcb",
                                         name=f"wff2_{m}_{q}")
                    nc.sync.dma_start(
                        out=wt,
                        in_=wpack[:, ff2 + (m * 4 + q) * 1024:ff2 + (m * 4 + q + 1) * 1024])
                    for k in range(NK):
                        nc.tensor.matmul(psf, wt[:, k * 128:(k + 1) * 128],
                                         h[q * NK + k],
                                         start=(q == 0 and k == 0),
                                         stop=(q == NM2 // NK - 1 and k == NK - 1))
                nc.vector.scalar_tensor_tensor(
                    y3[m], psf, bc[:, COFF["b_ff2"] + m:COFF["b_ff2"] + m + 1],
                    o2[m], ALU.add, ALU.add)
            o3 = [o3p.tile([128, R], F32, tag="o3", name=f"o3_{m}")
                  for m in range(NK)]
            _ln(nc, pools, y3, bc, COFF["g3"], COFF["be3"], o3, ones128)
            for m in range(NK):
                nc.sync.dma_start(out=out_t[m * 128:(m + 1) * 128, :], in_=o3[m])

    nc.compile()
    return nc


def _get_nc():
    if "nc" not in _NC_CACHE:
        _NC_CACHE["nc"] = build_nc()
    return _NC_CACHE["nc"]


def _pack_dd(w, nb, cb):
    # [D, nb*cb] weight -> [128, nb*NK*cb]: tile j cols [j*NK*cb:(j+1)*NK*cb],
    # within tile col k*cb + c = w[k*128 + p, j*cb + c]
    return w.reshape(NK, 128, nb, cb).transpose(1, 2, 0, 3).reshape(128, -1)


def _make_in_maps(inputs):
    import ml_dtypes
    BF = ml_dtypes.bfloat16
    f32 = np.float32

    wpack = np.zeros((128, W_COLS), dtype=BF)
    for nm, nb, cb in (("wq1", 8, 128), ("wk1", 8, 128), ("wv1", 2, 512),
                       ("wo1", 8, 128), ("wq2", 8, 128), ("wk2", 8, 128),
                       ("wv2", 2, 512), ("wo2", 8, 128)):
        blk = _pack_dd(np.asarray(inputs[nm], f32), nb, cb)
        wpack[:, WOFF[nm]:WOFF[nm] + 8192] = blk.astype(BF)
    ff1 = np.asarray(inputs["w_ff1"], f32).reshape(NK, 128, NM2, 128)
    wpack[:, WOFF["w_ff1"]:WOFF["w_ff1"] + 32768] = \
        ff1.transpose(1, 2, 0, 3).reshape(128, -1).astype(BF)
    ff2 = np.asarray(inputs["w_ff2"], f32).reshape(4, NK, 128, NK, 128)
    wpack[:, WOFF["w_ff2"]:WOFF["w_ff2"] + 32768] = \
        ff2.transpose(2, 3, 0, 1, 4).reshape(128, -1).astype(BF)
    wpack[0, WOFF["bv1"]:WOFF["bv1"] + 1024] = np.asarray(inputs["bv1"], f32).astype(BF)
    wpack[0, WOFF["bv2"]:WOFF["bv2"] + 1024] = np.asarray(inputs["bv2"], f32).astype(BF)

    cpack = np.zeros((128, C_COLS), dtype=f32)
    for nm in ("bq1", "bk1", "bo1", "bq2", "bk2", "bo2", "b_ff2",
               "g1", "be1", "g2", "be2", "g3", "be3"):
        cpack[:, COFF[nm]:COFF[nm] + 8] = np.asarray(inputs[nm], f32).reshape(8, 128).T
    cpack[:, COFF["b_ff1"]:COFF["b_ff1"] + 32] = \
        np.asarray(inputs["b_ff1"], f32).reshape(32, 128).T

    full_k = np.arange(S)
    in_maps = []
    metas = []
    for c in range(8):
        b, half = c // 2, c % 2
        qidx = np.concatenate([np.arange(128) + 128 * blk for blk in BLOCKS[half]])
        apack = np.zeros((D, A_COLS), dtype=BF)
        xt_b = np.asarray(inputs["inputs"][b], f32).T
        apack[:, A_XT:A_XT + S] = xt_b.astype(BF)
        apack[:, A_XQ:A_XQ + R] = xt_b[:, qidx].astype(BF)
        apack[:, A_ENC:A_ENC + S] = np.asarray(inputs["enc_outputs"][b], f32).T.astype(BF)
        for s in range(NK):
            c0 = 128 * (s // 2)
            blkm = np.where(full_k[s * 128:(s + 1) * 128, None] <= qidx[None, c0:c0 + 128],
                            0.0, NEG)
            apack[0:128, A_MASK + s * 128:A_MASK + (s + 1) * 128] = blkm.astype(BF)
        in_maps.append({"apack": apack, "wpack": wpack, "cpack": cpack})
        metas.append((b, qidx))
    return in_maps, metas


def kernel(**inputs):
    nc = _get_nc()
    in_maps, metas = _make_in_maps(inputs)
    res = run_bass_kernel_spmd(nc, in_maps, core_ids=list(range(8)))
    out = np.zeros((B, S, D), dtype=np.float32)
    for c, (b, qidx) in enumerate(metas):
        out[b, qidx, :] = res.results[c]["out_t"].T
    return out


# revision 21
# speedup vs baseline: 1.8539x; 1.0057x over previous
"""Fused transformer decoder layer (self-attn + cross-attn + FFN, 3 LayerNorms)
for Trainium2, SPMD across 8 NeuronCores.

Sharding: 2 cores per batch element (B=4). Each core owns 512 query rows of
its batch element, picked as four 128-row blocks interleaved so the causal
self-attention work is balanced across the pair ({0,3,4,7} / {1,2,5,6}).
K/V projections are computed redundantly per core (no collectives needed).

On-device layout: activations are kept feature-major ("transposed", [D, rows])
so every linear layer uses the weight matrices exactly as stored:
    out^T [Dout, r] = matmul(lhsT=W[D, Dout]-tile, rhs=act^T[D, r]-tile).
Attention scores are computed transposed (scores^T[k, q] = K.Q^T per head);
softmax denominators are harvested by augmenting V with 64 all-ones columns,
which lands the per-query sums partition-replicated next to the attnV output.

All matmul operands are bf16 (fp32 PSUM accumulation); LayerNorm statistics
run in fp32. Host-side, the inputs are packed into just 3 DRAM tensors to
minimize per-call dispatch overhead:
  apack [D, 3584] bf16 : xt^T | xq^T | enc^T | causal-mask blocks
  wpack [128, 133120] bf16 : all 10 weight matrices pre-tiled so every
                             weight DMA is a dense [128, ncols] column slice
  cpack [128, 136] f32 : bias / gamma / beta columns
"""

import numpy as np

import concourse.bacc as bacc
import concourse.bass as bass
import concourse.mybir as mybir
import concourse.tile as tile
from concourse.bass_utils import run_bass_kernel_spmd

F32 = mybir.dt.float32
F32R = mybir.dt.float32r
BF16 = mybir.dt.bfloat16
AF = mybir.ActivationFunctionType
ALU = mybir.AluOpType

B, S, D, DFF, H = 4, 1024, 1024, 4096, 16
R = 512                   # query rows per core
NK = D // 128             # 8 k-tiles over D
NP = H // 2               # 8 head pairs
NM2 = DFF // 128          # 32 m-tiles over DFF
EPS = 1e-3
NEG = -1e9
BLOCKS = [[0, 3, 4, 7], [1, 2, 5, 6]]   # 128-row q-blocks per half-core

# ---- packed-layout offsets (columns) ----
# apack columns
A_XT, A_XQ, A_ENC, A_MASK = 0, 1024, 1536, 2560
A_COLS = 3584
# wpack columns: each D x D weight is 8 tiles x 1024 cols (wv: 4 x 2048)
WOFF = {"wq1": 0, "wk1": 8192, "wv1": 16384, "wo1": 24576,
        "wq2": 32768, "wk2": 40960, "wv2": 49152, "wo2": 57344,
        "w_ff1": 65536, "w_ff2": 98304, "bv1": 131072, "bv2": 132096}
W_COLS = 133120
# cpack columns
COFF = {"bq1": 0, "bk1": 8, "bo1": 16, "bq2": 24, "bk2": 32, "bo2": 40,
        "b_ff2": 48, "g1": 56, "be1": 64, "g2": 72, "be2": 80,
        "g3": 88, "be3": 96, "b_ff1": 104}
C_COLS = 136

_NC_CACHE = {}


def _ln(nc, pools, y, bc, g_off, be_off, out_tiles, ones128):
    """LayerNorm over the partition (feature) axis of 8 [128, R] f32 tiles."""
    ps, tmp = pools["psum"], pools["lntmp"]
    pssum = ps.tile([128, R], F32, tag="ps_gen", name="ln_ps_sum")
    for m in range(NK):
        nc.tensor.matmul(pssum, ones128, y[m], start=(m == 0), stop=(m == NK - 1))
    pssq = ps.tile([128, R], F32, tag="ps_gen", name="ln_ps_sq")
    for m in range(NK):
        sq = pools["sq"].tile([128, R], F32R, tag="ln_sqt", name="ln_sqt")
        nc.scalar.activation(sq, y[m].bitcast(F32), AF.Square)
        nc.tensor.matmul(pssq, ones128, sq, start=(m == 0), stop=(m == NK - 1))
    mean = tmp.tile([128, R], F32, tag="ln_mean", name="ln_mean")
    nc.vector.tensor_scalar_mul(mean, pssum, 1.0 / D)
    rv = tmp.tile([128, R], F32, tag="ln_sc", name="ln_rv")
    nc.vector.tensor_scalar_mul(rv, pssq, 1.0 / D)      # E[x^2]
    msq = tmp.tile([128, R], F32, tag="ln_t", name="ln_msq")
    nc.vector.tensor_mul(msq, mean, mean)
    nc.vector.tensor_sub(rv, rv, msq)                   # var
    nc.scalar.activation(rv, rv, AF.Sqrt, bias=pools["epsc"][:, 0:1])
    nc.vector.reciprocal(rv, rv)                        # rstd (broadcast)
    nc.vector.tensor_mul(mean, mean, rv)                # mean*rstd (broadcast)
    for m in range(NK):
        t = tmp.tile([128, R], F32, tag="ln_t", name="ln_t")
        nc.vector.tensor_mul(t, y[m].bitcast(F32), rv)
        nc.vector.tensor_sub(t, t, mean)
        nc.vector.tensor_scalar(out_tiles[m], t, bc[:, g_off + m:g_off + m + 1],
                                bc[:, be_off + m:be_off + m + 1],
                                ALU.mult, ALU.add)


def _attention(nc, pools, wpack, bc, q_src, kv_src, resid, wq_off, wk_off,
               wv_off, wo_off, bq_off, bk_off, bv_off, bo_off,
               mask_sb, y_out, ones_bv):
    """One multi-head attention block + residual; writes pre-LN y_out tiles.

    q_src/kv_src/resid: bf16 [128, *] tiles. y_out: f32 [128, R] tiles.
    """
    wpool, ps, tmp = pools["w"], pools["psum"], pools["atmp"]
    causal = mask_sb is not None

    # persistent V_aug tiles (one per pair in the current 4-pair group);
    # ones columns written once per attention
    vaugs = [pools["vaug"].tile([128, NK, 256], BF16, tag=f"vaug_{i}",
                                name=f"vaug_{i}") for i in range(4)]
    for i in range(4):
        nc.vector.memset(vaugs[i][:, :, 64:192], 1.0)

    nh = []
    for p in range(NP):
        # ---- V for four head pairs produced together (moving dim 512) ----
        # V_aug[:, s, :]: cols [v_h0(64) | ones(128) | v_h1(64)]
        if p % 4 == 0:
            g = p // 4
            wvb = wpool.tile([128, 4096], BF16, tag="wvb", name=f"wvb_{g}")
            nc.sync.dma_start(out=wvb,
                              in_=wpack[:, wv_off + g * 4096:wv_off + (g + 1) * 4096])
            bv_chunk = tmp.tile([1, 512], BF16, tag="bv_chunk", name=f"bvc_{g}")
            nc.sync.dma_start(out=bv_chunk,
                              in_=wpack[0:1, bv_off + g * 512:bv_off + (g + 1) * 512])
            # partition-replicated bias tile, one K=1 matmul per group
            psB = ps.tile([128, 512], F32, tag="ps_gen", name="ps_bv")
            nc.tensor.matmul(psB, ones_bv[0:1, :], bv_chunk[0:1, :],
                             start=True, stop=True)
            brep = tmp.tile([128, 512], F32, tag="brep", name=f"brep_{g}")
            nc.vector.tensor_copy(brep, psB)
            for s in range(NK):
                psv = ps.tile([128, 512], F32, tag="ps_gen", name="ps_v")
                for k in range(NK):
                    nc.tensor.matmul(psv, kv_src[k][:, s * 128:(s + 1) * 128],
                                     wvb[:, k * 512:(k + 1) * 512],
                                     start=(k == 0), stop=(k == NK - 1))
                for i in range(4):
                    dst = vaugs[i][:, s, :].rearrange(
                        "p (a c) -> p a c", c=64)[:, 0:4:3, :]
                    srcp = psv[:, i * 128:(i + 1) * 128].rearrange(
                        "p (a c) -> p a c", c=64)
                    brp = brep[:, i * 128:(i + 1) * 128].rearrange(
                        "p (a c) -> p a c", c=64)
                    nc.vector.tensor_tensor(dst, srcp, brp, ALU.add)
        vaug = vaugs[p % 4]

        # ---- Q^T then K^T for this pair (ACT latencies hide under K matmuls) ----
        wcb = wpool.tile([128, 1024], BF16, tag="wcb", name=f"wqc_{p}")
        nc.sync.dma_start(out=wcb,
                          in_=wpack[:, wq_off + p * 1024:wq_off + (p + 1) * 1024])
        psq = ps.tile([128, R], F32, tag="ps_gen", name="ps_q")
        for k in range(NK):
            nc.tensor.matmul(psq, wcb[:, k * 128:(k + 1) * 128], q_src[k],
                             start=(k == 0), stop=(k == NK - 1))
        qt_p = pools["qt"].tile([128, R], BF16, tag="qt", name=f"qt_{p}")
        nc.scalar.activation(qt_p, psq, AF.Identity,
                             bias=bc[:, bq_off + p:bq_off + p + 1])
        wcb = wpool.tile([128, 1024], BF16, tag="wcb", name=f"wkc_{p}")
        nc.sync.dma_start(out=wcb,
                          in_=wpack[:, wk_off + p * 1024:wk_off + (p + 1) * 1024])
        kt_p = pools["kt"].tile([128, S], BF16, tag="kt", name=f"kt_{p}")
        psk0 = ps.tile([128, R], F32, tag="ps_gen", name="ps_k0")
        psk1 = ps.tile([128, R], F32, tag="ps_gen", name="ps_k1")
        for k in range(NK):
            # both halves share the stationary -> one weight load per k
            nc.tensor.matmul(psk0, wcb[:, k * 128:(k + 1) * 128],
                             kv_src[k][:, 0:512],
                             start=(k == 0), stop=(k == NK - 1),
                             skip_group_check=True)
            nc.tensor.matmul(psk1, wcb[:, k * 128:(k + 1) * 128],
                             kv_src[k][:, 512:1024],
                             start=(k == 0), stop=(k == NK - 1),
                             skip_group_check=True)
        nc.scalar.activation(kt_p[:, 0:512], psk0,
                             AF.Identity, bias=bc[:, bk_off + p:bk_off + p + 1])
        nc.scalar.activation(kt_p[:, 512:1024], psk1,
                             AF.Identity, bias=bc[:, bk_off + p:bk_off + p + 1])

        # ---- scores^T, exp, attnV (softmax sums ride along in V_aug ones) ----
        psa0 = pools["psatt"].tile([128, R], F32, tag="ps_att0", name="ps_att0")
        psa1 = pools["psatt"].tile([128, R], F32, tag="ps_att1", name="ps_att1")
        pend = None
        for s in range(NK):
            c0 = 128 * (s // 2) if causal else 0
            n = R - c0
            ssc0 = pools["pssc"].tile([128, R], F32, tag="ps_sc0", name="ps_sc0")
            ssc1 = pools["pssc"].tile([128, R], F32, tag="ps_sc1", name="ps_sc1")
            nc.tensor.matmul(ssc0[:, 0:n], kt_p[0:64, s * 128:(s + 1) * 128],
                             qt_p[0:64, c0:R], start=True, stop=True)
            nc.tensor.matmul(ssc1[:, 0:n], kt_p[64:128, s * 128:(s + 1) * 128],
                             qt_p[64:128, c0:R], start=True, stop=True)
            if causal:
                nc.vector.tensor_add(ssc0[:, 0:128], ssc0[:, 0:128],
                                     mask_sb[:, s * 128:(s + 1) * 128])
                nc.vector.tensor_add(ssc1[:, 0:128], ssc1[:, 0:128],
                                     mask_sb[:, s * 128:(s + 1) * 128])
            e0 = pools["exp"].tile([128, R], BF16, tag="e0", name="e0")
            e1 = pools["exp"].tile([128, R], BF16, tag="e1", name="e1")
            nc.scalar.activation(e0[:, 0:n], ssc0[:, 0:n], AF.Exp, scale=0.125)
            nc.scalar.activation(e1[:, 0:n], ssc1[:, 0:n], AF.Exp, scale=0.125)
            if pend is not None:
                pc0, pn, pe0, pe1, pss = pend
                nc.tensor.matmul(psa0[:, pc0:R], vaug[:, pss, 0:128], pe0[:, 0:pn],
                                 start=(pss == 0), stop=False, skip_group_check=True)
                nc.tensor.matmul(psa1[:, pc0:R], vaug[:, pss, 128:256], pe1[:, 0:pn],
                                 start=(pss == 0), stop=False, skip_group_check=True)
            pend = (c0, n, e0, e1, s)
        pc0, pn, pe0, pe1, pss = pend
        nc.tensor.matmul(psa0[:, pc0:R], vaug[:, pss, 0:128], pe0[:, 0:pn],
                         start=False, stop=True, skip_group_check=True)
        nc.tensor.matmul(psa1[:, pc0:R], vaug[:, pss, 128:256], pe1[:, 0:pn],
                         start=False, stop=True, skip_group_check=True)

        # normalize: head0 out rows 0:64 / sums 64:128; head1 sums 0:64 / out 64:128
        nh_p = pools["nh"].tile([128, R], BF16, tag=f"nh_{p}", name=f"nh_{p}")
        inv = tmp.tile([128, R], F32, tag="inv", name="inv")
        invs = tmp.tile([128, R], F32, tag="invs", name="invs")
        nc.vector.reciprocal(inv[64:128, :], psa0[64:128, :])
        nc.sync.dma_start(out=invs[0:64, :], in_=inv[64:128, :])
        nc.vector.tensor_mul(nh_p[0:64, :], psa0[0:64, :], invs[0:64, :])
        nc.vector.reciprocal(inv[0:64, :], psa1[0:64, :])
        nc.sync.dma_start(out=invs[64:128, :], in_=inv[0:64, :])
        nc.vector.tensor_mul(nh_p[64:128, :], psa1[64:128, :], invs[64:128, :])
        nh.append(nh_p)

    # ---- output projection + bias + residual ----
    for m in range(NK):
        wcb = wpool.tile([128, 1024], BF16, tag="wcb", name=f"woc_{m}")
        nc.sync.dma_start(out=wcb,
                          in_=wpack[:, wo_off + m * 1024:wo_off + (m + 1) * 1024])
        pso = ps.tile([128, R], F32, tag="ps_gen", name="ps_o")
        for p in range(NP):
            nc.tensor.matmul(pso, wcb[:, p * 128:(p + 1) * 128], nh[p],
                             start=(p == 0), stop=(p == NP - 1))
        nc.vector.scalar_tensor_tensor(y_out[m], pso, bc[:, bo_off + m:bo_off + m + 1],
                                       resid[m], ALU.add, ALU.add)


def build_nc():
    nc = bacc.Bacc("TRN2", target_bir_lowering=False, debug=False)

    apack = nc.dram_tensor("apack", [D, A_COLS], BF16, kind="ExternalInput")
    wpack = nc.dram_tensor("wpack", [128, W_COLS], BF16, kind="ExternalInput")
    cpack = nc.dram_tensor("cpack", [128, C_COLS], F32, kind="ExternalInput")
    out_t = nc.dram_tensor("out_t", [D, R], F32, kind="ExternalOutput")

    from contextlib import ExitStack
    with tile.TileContext(nc) as tc, ExitStack() as ctx:
        pools = {
            "const": ctx.enter_context(tc.tile_pool(name="const", bufs=1)),
            "w": ctx.enter_context(tc.tile_pool(name="wpool", bufs=4)),
            "psum": ctx.enter_context(tc.tile_pool(name="pspool", bufs=2, space="PSUM")),
            "lntmp": ctx.enter_context(tc.tile_pool(name="lntmp", bufs=1)),
            "sq": ctx.enter_context(tc.tile_pool(name="sqpool", bufs=2)),
            "o2p": ctx.enter_context(tc.tile_pool(name="o2pool", bufs=1)),
        }
        const = pools["const"]

        ones_f = const.tile([128, 128], F32, tag="ones_f", name="ones_f")
        nc.vector.memset(ones_f, 1.0)
        ones128 = const.tile([128, 128], F32R, tag="ones128", name="ones128")
        nc.vector.tensor_copy(ones128, ones_f)
        ones_bv = const.tile([128, 128], BF16, tag="ones_bv", name="ones_bv")
        nc.vector.memset(ones_bv, 1.0)
        epsc = const.tile([128, 1], F32, tag="epsc", name="epsc")
        nc.vector.memset(epsc, EPS)
        pools["epsc"] = epsc
        bc = const.tile([128, C_COLS], F32, tag="bc", name="bc")
        nc.sync.dma_start(out=bc, in_=cpack[:, :])

        o2 = [pools["o2p"].tile([128, R], BF16, tag=f"o2_{m}", name=f"o2_{m}")
              for m in range(NK)]

        # ================= attention scope =================
        with ExitStack() as actx:
            for nm, bufs, space in (("acts", 1, "SBUF"), ("qt", 3, "SBUF"),
                                    ("kt", 3, "SBUF"), ("vaug", 2, "SBUF"),
                                    ("nh", 1, "SBUF"), ("exp", 3, "SBUF"),
                                    ("atmp", 1, "SBUF"), ("amask", 1, "SBUF"),
                                    ("pssc", 2, "PSUM"), ("psatt", 1, "PSUM")):
                pools[nm] = actx.enter_context(
                    tc.tile_pool(name=nm, bufs=bufs, space=space))
            acts = pools["acts"]

            xt_sb = [acts.tile([128, S], BF16, tag=f"kv_{k}", name=f"xt_{k}")
                     for k in range(NK)]
            xq_sb = [acts.tile([128, R], BF16, tag=f"xq_{k}", name=f"xq_{k}")
                     for k in range(NK)]
            for k in range(NK):
                nc.sync.dma_start(out=xt_sb[k][:, 0:512],
                                  in_=apack[k * 128:(k + 1) * 128, A_XT:A_XT + 512])
            for k in range(NK):
                nc.sync.dma_start(out=xt_sb[k][:, 512:1024],
                                  in_=apack[k * 128:(k + 1) * 128, A_XT + 512:A_XT + S])
            for k in range(NK):
                nc.sync.dma_start(out=xq_sb[k],
                                  in_=apack[k * 128:(k + 1) * 128, A_XQ:A_XQ + R])
            mask_sb = pools["amask"].tile([128, 1024], BF16, tag="mask", name="mask")
            nc.sync.dma_start(out=mask_sb, in_=apack[0:128, A_MASK:A_MASK + 1024])

            y1 = [acts.tile([128, R], F32R, tag=f"y_{m}", name=f"y1_{m}")
                  for m in range(NK)]
            _attention(nc, pools, wpack, bc, xq_sb, xt_sb, xq_sb,
                       WOFF["wq1"], WOFF["wk1"], WOFF["wv1"], WOFF["wo1"],
                       COFF["bq1"], COFF["bk1"], WOFF["bv1"], COFF["bo1"],
                       mask_sb, y1, ones_bv)
            enc_sb = [acts.tile([128, S], BF16, tag=f"env_{k}", name=f"enc_{k}")
                      for k in range(NK)]
            for k in range(NK):
                nc.sync.dma_start(out=enc_sb[k],
                                  in_=apack[k * 128:(k + 1) * 128, A_ENC:A_ENC + S])
            o1 = [acts.tile([128, R], BF16, tag=f"xq_{m}", name=f"o1_{m}")
                  for m in range(NK)]
            _ln(nc, pools, y1, bc, COFF["g1"], COFF["be1"], o1, ones128)

            y2 = [acts.tile([128, R], F32R, tag=f"y_{m}", name=f"y2_{m}")
                  for m in range(NK)]
            _attention(nc, pools, wpack, bc, o1, enc_sb, o1,
                       WOFF["wq2"], WOFF["wk2"], WOFF["wv2"], WOFF["wo2"],
                       COFF["bq2"], COFF["bk2"], WOFF["bv2"], COFF["bo2"],
                       None, y2, ones_bv)
            _ln(nc, pools, y2, bc, COFF["g2"], COFF["be2"], o2, ones128)

        # ================= FFN scope =================
        with ExitStack() as fctx:
            hpool = fctx.enter_context(tc.tile_pool(name="hpool", bufs=1))
            facts = fctx.enter_context(tc.tile_pool(name="facts", bufs=1))
            o3p = fctx.enter_context(tc.tile_pool(name="o3pool", bufs=2))

            ff1 = WOFF["w_ff1"]
            h = []
            for m in range(NM2):
                wt = pools["w"].tile([128, 1024], BF16, tag="wcb",
                                     name=f"wff1_{m}")
                nc.sync.dma_start(out=wt,
                                  in_=wpack[:, ff1 + m * 1024:ff1 + (m + 1) * 1024])
                psh = pools["psum"].tile([128, R], F32, tag="ps_gen", name="ps_h")
                for k in range(NK):
                    nc.tensor.matmul(psh, wt[:, k * 128:(k + 1) * 128], o2[k],
                                     start=(k == 0), stop=(k == NK - 1))
                h_m = hpool.tile([128, R], BF16, tag=f"h_{m}", name=f"h_{m}")
                nc.scalar.activation(h_m, psh, AF.Relu,
                                     bias=bc[:, COFF["b_ff1"] + m:COFF["b_ff1"] + m + 1])
                h.append(h_m)

            ff2 = WOFF["w_ff2"]
            y3 = [facts.tile([128, R], F32R, tag=f"y3_{m}", name=f"y3_{m}")
                  for m in range(NK)]
            for m in range(NK):
                psf = pools["psum"].tile([128, R], F32, tag="ps_gen", name="ps_f")
                for q in range(NM2 // NK):
                    wt = pools["w"].tile([128, 1024], BF16, tag="wcb",
                                         name=f"wff2_{m}_{q}")
                    nc.sync.dma_start(
                        out=wt,
                        in_=wpack[:, ff2 + (m * 4 + q) * 1024:ff2 + (m * 4 + q + 1) * 1024])
                    for k in range(NK):
                        nc.tensor.matmul(psf, wt[:, k * 128:(k + 1) * 128],
                                         h[q * NK + k],
                                         start=(q == 0 and k == 0),
                                         stop=(q == NM2 // NK - 1 and k == NK - 1))
                nc.vector.scalar_tensor_tensor(
                    y3[m], psf, bc[:, COFF["b_ff2"] + m:COFF["b_ff2"] + m + 1],
                    o2[m], ALU.add, ALU.add)
            o3 = [o3p.tile([128, R], F32, tag="o3", name=f"o3_{m}")
                  for m in range(NK)]
            _ln(nc, pools, y3, bc, COFF["g3"], COFF["be3"], o3, ones128)
            for m in range(NK):
                nc.sync.dma_start(out=out_t[m * 128:(m + 1) * 128, :], in_=o3[m])

    nc.compile()
    return nc


def _get_nc():
    if "nc" not in _NC_CACHE:
        _NC_CACHE["nc"] = build_nc()
    return _NC_CACHE["nc"]


def _pack_dd(w, nb, cb):
    # [D, nb*cb] weight -> [128, nb*NK*cb]: tile j cols [j*NK*cb:(j+1)*NK*cb],
    # within tile col k*cb + c = w[k*128 + p, j*cb + c]
    return w.reshape(NK, 128, nb, cb).transpose(1, 2, 0, 3).reshape(128, -1)


def _make_in_maps(inputs):
    import ml_dtypes
    BF = ml_dtypes.bfloat16
    f32 = np.float32

    wpack = np.zeros((128, W_COLS), dtype=BF)
    for nm, nb, cb in (("wq1", 8, 128), ("wk1", 8, 128), ("wv1", 2, 512),
                       ("wo1", 8, 128), ("wq2", 8, 128), ("wk2", 8, 128),
                       ("wv2", 2, 512), ("wo2", 8, 128)):
        blk = _pack_dd(np.asarray(inputs[nm], f32), nb, cb)
        wpack[:, WOFF[nm]:WOFF[nm] + 8192] = blk.astype(BF)
    ff1 = np.asarray(inputs["w_ff1"], f32).reshape(NK, 128, NM2, 128)
    wpack[:, WOFF["w_ff1"]:WOFF["w_ff1"] + 32768] = \
        ff1.transpose(1, 2, 0, 3).reshape(128, -1).astype(BF)
    ff2 = np.asarray(inputs["w_ff2"], f32).reshape(4, NK, 128, NK, 128)
    wpack[:, WOFF["w_ff2"]:WOFF["w_ff2"] + 32768] = \
        ff2.transpose(2, 3, 0, 1, 4).reshape(128, -1).astype(BF)
    wpack[0, WOFF["bv1"]:WOFF["bv1"] + 1024] = np.asarray(inputs["bv1"], f32).astype(BF)
    wpack[0, WOFF["bv2"]:WOFF["bv2"] + 1024] = np.asarray(inputs["bv2"], f32).astype(BF)

    cpack = np.zeros((128, C_COLS), dtype=f32)
    for nm in ("bq1", "bk1", "bo1", "bq2", "bk2", "bo2", "b_ff2",
               "g1", "be1", "g2", "be2", "g3", "be3"):
        cpack[:, COFF[nm]:COFF[nm] + 8] = np.asarray(inputs[nm], f32).reshape(8, 128).T
    cpack[:, COFF["b_ff1"]:COFF["b_ff1"] + 32] = \
        np.asarray(inputs["b_ff1"], f32).reshape(32, 128).T

    full_k = np.arange(S)
    in_maps = []
    metas = []
    for c in range(8):
        b, half = c // 2, c % 2
        qidx = np.concatenate([np.arange(128) + 128 * blk for blk in BLOCKS[half]])
        apack = np.zeros((D, A_COLS), dtype=BF)
        xt_b = np.asarray(inputs["inputs"][b], f32).T
        apack[:, A_XT:A_XT + S] = xt_b.astype(BF)
        apack[:, A_XQ:A_XQ + R] = xt_b[:, qidx].astype(BF)
        apack[:, A_ENC:A_ENC + S] = np.asarray(inputs["enc_outputs"][b], f32).T.astype(BF)
        for s in range(NK):
            c0 = 128 * (s // 2)
            blkm = np.where(full_k[s * 128:(s + 1) * 128, None] <= qidx[None, c0:c0 + 128],
                            0.0, NEG)
            apack[0:128, A_MASK + s * 128:A_MASK + (s + 1) * 128] = blkm.astype(BF)
        in_maps.append({"apack": apack, "wpack": wpack, "cpack": cpack})
        metas.append((b, qidx))
    return in_maps, metas


def kernel(**inputs):
    nc = _get_nc()
    in_maps, metas = _make_in_maps(inputs)
    res = run_bass_kernel_spmd(nc, in_maps, core_ids=list(range(8)))
    out = np.zeros((B, S, D), dtype=np.float32)
    for c, (b, qidx) in enumerate(metas):
        out[b, qidx, :] = res.results[c]["out_t"].T
    return out
